# revision 2
# baseline (speedup 1.0000x reference)
"""DeltaNet-style chunked delta-rule block on 8 Trainium2 NeuronCores.

Sharding: data-parallel over batch B=8 (one batch element per core); the
sequential inter-chunk scan stays local per core. Each core runs the same
SPMD program:

  pass A: per 128-row chunk-pair: X^T via PE transpose, Q/K projections,
          beta, chunk-local T = (I - tril_strict(Kb K^T))^{-1} via binary
          lifting, W^T = Kb^T T^T; spills a combined per-group operand
          block [wT0|qT0|wT1|qT1|kT1|kT0], K natural, T^T, beta to DRAM.
  pass B: V projection, U = T @ Vb, spilled to scratch.
  phase 2: sequential scan over chunks in groups of 2 (both chunks read
           the group's incoming state S; the second chunk gets explicit
           rank-C corrections), fused with the output projection.

All matmuls run in float32r (full-speed PE path for moving dim >= 256;
operands must be written pre-rounded, so matmul operand tiles are
float32r, produced by DVE/ACT copies). Everything else is float32.
"""
import contextlib

import numpy as np

import concourse.bass as bass
import concourse.mybir as mybir
import concourse.tile as tile
from concourse import bacc
from concourse.bass_utils import run_bass_kernel_spmd
from concourse.masks import make_identity

FP = mybir.dt.float32
FPR = mybir.dt.float32r
AL = mybir.AluOpType

B, L, D, C = 8, 4096, 1024, 64
NCH = L // C          # 64 chunks
NPAIR = NCH // 2      # 32 chunk pairs / scan groups
KT = D // 128         # 8 contraction k-tiles
NH = D // 512         # 2 moving-dim halves

# offsets in the combined per-group operand block (per k-tile, 384 wide)
_W0, _Q0, _W1, _Q1, _K1, _K0 = 0, 64, 128, 192, 256, 320

_compiled = {}


def _transpose_weight(nc, wT_tiles, w_ap, wnat_pool, ps_pool, ident, ps_tag):
    """wT_tiles[jt][:, it*128:(it+1)*128] = W[it-block, jt-block]^T."""
    for itb in range(KT // 4):
        wns = []
        for i4 in range(4):
            it = itb * 4 + i4
            wn = wnat_pool.tile([128, D], FP, tag=f"wnat{i4}", name=f"wnat{i4}")
            nc.sync.dma_start(out=wn[:], in_=w_ap[it * 128:(it + 1) * 128, :])
            wns.append(wn)
        for jt in range(KT):
            ps = ps_pool.tile([128, 512], FP, tag=ps_tag, name="wt_ps")
            for i4 in range(4):
                nc.tensor.transpose(ps[:, i4 * 128:(i4 + 1) * 128],
                                    wns[i4][:, jt * 128:(jt + 1) * 128], ident[:])
            nc.scalar.copy(out=wT_tiles[jt][:, itb * 512:(itb + 1) * 512], in_=ps[:])


def _replicate_bias(nc, pool, b_ap, tag):
    t = pool.tile([128, D], FP, tag=tag)
    bc = bass.AP(tensor=b_ap.tensor, offset=b_ap.offset,
                 ap=[[0, 128]] + list(b_ap.ap[1:]))
    nc.sync.dma_start(out=t[:], in_=bc)
    return t


def _build(reps=1, phases='full'):
    nc = bacc.Bacc("TRN2", target_bir_lowering=False, debug=False)

    x_d = nc.dram_tensor("x", [L, D], FP, kind="ExternalInput").ap()
    w_d, b_d = {}, {}
    for nm in ("wq", "wk", "wv", "wo"):
        w_d[nm] = nc.dram_tensor(nm, [D, D], FP, kind="ExternalInput").ap()
        b_d[nm] = nc.dram_tensor("b" + nm[1], [1, D], FP, kind="ExternalInput").ap()
    out_d = nc.dram_tensor("out", [L, D], FP, kind="ExternalOutput").ap()
    x2_d = nc.dram_tensor("x2", [L, D], FP).ap() if reps > 1 else None

    # DRAM scratch (per-core private)
    grp_s = nc.dram_tensor("grp_scr", [NPAIR, 128, KT, 384], FPR).ap()
    kn_s = nc.dram_tensor("kn_scr", [L, D], FPR).ap()                # K natural
    u_s = nc.dram_tensor("u_scr", [NCH, 64, D], FPR).ap()            # U = T @ Vb
    tt_s = nc.dram_tensor("tt_scr", [NCH, 64, 64], FPR).ap()         # T^T
    at_s = nc.dram_tensor("at_scr", [NCH, 64, 64], FPR).ap()         # A^T masked
    kq_s = nc.dram_tensor("kq_scr", [NPAIR, 128, KT, 256], FPR).ap()  # [kT|qT] pair
    be_s = nc.dram_tensor("be_scr", [L, 1], FP).ap()                 # beta

    with tile.TileContext(nc) as tc, contextlib.ExitStack() as ctx:
      consts = ctx.enter_context(tc.tile_pool(name="consts", bufs=1))
      for rep in range(reps):
        if rep > 0:
            # serialize rep boundaries: x2 = copy of x, gated behind rep-1's
            # last-written tensor
            if phases == 'a':
                gate_src = grp_s[NPAIR - 1, 127:128, KT - 1, 0:4].bitcast(FP)
            elif phases == 'ab':
                gate_src = u_s[NCH - 1, 63:64, 0:4].bitcast(FP)
            else:
                gate_src = out_d[L - 1:L, 0:4]
            nc.sync.dma_start(out=x2_d[0:1, 0:4], in_=gate_src)
            nc.sync.dma_start(out=x2_d[:], in_=x_d[:])
            x_cur = x2_d
        else:
            x_cur = x_d
        ident = consts.tile([128, 128], FP, tag="ident", name="ident")
        make_identity(nc, ident[:])
        ident_r = consts.tile([128, 128], FPR, tag="identr", name="identr")
        nc.vector.tensor_copy(out=ident_r[:], in_=ident[:])
        mask_l = consts.tile([128, 64], FP, tag="maskl", name="maskl")
        nc.gpsimd.memset(mask_l[:], -1.0)
        for _mh in range(2):
            nc.gpsimd.affine_select(out=mask_l[_mh * 64:(_mh + 1) * 64, :],
                                    in_=mask_l[_mh * 64:(_mh + 1) * 64, :],
                                    compare_op=AL.is_ge, fill=0.0, base=-1,
                                    pattern=[[-1, 64]], channel_multiplier=1)
        mask_ui = consts.tile([64, 64], FP, tag="maskui", name="maskui")
        nc.gpsimd.memset(mask_ui[:], 1.0)
        nc.gpsimd.affine_select(out=mask_ui[:], in_=mask_ui[:], compare_op=AL.is_ge,
                                fill=0.0, base=0, pattern=[[1, 64]],
                                channel_multiplier=-1)

        # ============ pass A1: Q/K projections + transposes (dense) =========
        with tc.tile_pool(name="wA", bufs=1) as wA, \
             tc.tile_pool(name="wnatA", bufs=2) as wnatA, \
             tc.tile_pool(name="pA", bufs=2) as pA, \
             tc.tile_pool(name="pAc", bufs=2) as pAc, \
             tc.tile_pool(name="pA_big", bufs=6, space="PSUM") as bigps:
            wT = {nm: [wA.tile([128, D], FPR, tag=f"wT_{nm}_{jt}", name=f"wT_{nm}_{jt}")
                       for jt in range(KT)] for nm in ("wq", "wk")}
            _transpose_weight(nc, wT["wq"], w_d["wq"], wnatA, bigps, ident, "big")
            _transpose_weight(nc, wT["wk"], w_d["wk"], wnatA, bigps, ident, "big")
            bias_q = _replicate_bias(nc, wA, b_d["wq"], "bias_q")
            bias_k = _replicate_bias(nc, wA, b_d["wk"], "bias_k")

            for p in range(NPAIR):
                x_sb = pA.tile([128, D], FP, tag="x")
                nc.sync.dma_start(out=x_sb[:], in_=x_cur[p * 128:(p + 1) * 128, :])
                xt_sb = pA.tile([128, KT, 128], FPR, tag="xt")
                for jtb in range(KT // 4):
                    ps = bigps.tile([128, 512], FP, tag="big", name="xt_ps")
                    for j4 in range(4):
                        jt = jtb * 4 + j4
                        nc.tensor.transpose(ps[:, j4 * 128:(j4 + 1) * 128],
                                            x_sb[:, jt * 128:(jt + 1) * 128],
                                            ident[:])
                    nc.scalar.copy(
                        out=xt_sb[:, jtb * 4:(jtb + 1) * 4, :],
                        in_=ps[:].rearrange("p (a b) -> p a b", b=128))

                q_pair = pAc.tile([128, D], FPR, tag="qpair")
                k_pair = pAc.tile([128, D], FPR, tag="kpair")
                for nm, dst, bias in (("wk", k_pair, bias_k), ("wq", q_pair, bias_q)):
                    for h in range(NH):
                        hs = slice(h * 512, (h + 1) * 512)
                        pp = bigps.tile([128, 512], FP, tag="big", name="proj_ps")
                        for jt in range(KT):
                            nc.tensor.matmul(pp[:], xt_sb[:, jt, :],
                                             wT[nm][jt][:, hs],
                                             start=(jt == 0), stop=(jt == KT - 1))
                        nc.vector.tensor_tensor(out=dst[:, hs], in0=pp[:],
                                                in1=bias[:, hs], op=AL.add)
                nc.sync.dma_start(out=kn_s[p * 128:(p + 1) * 128, :], in_=k_pair[:])

                # beta for both chunks
                tmp = pAc.tile([128, D], FP, tag="tmp", bufs=1)
                nc.vector.tensor_tensor(out=tmp[:], in0=k_pair[:], in1=k_pair[:],
                                        op=AL.mult)
                beta = pAc.tile([128, 1], FP, tag="beta")
                nc.vector.reduce_sum(out=beta[:], in_=tmp[:], axis=mybir.AxisListType.X)
                nc.vector.tensor_scalar(out=beta[:], in0=beta[:], scalar1=1e-6,
                                        scalar2=None, op0=AL.add)
                nc.vector.reciprocal(out=beta[:], in_=beta[:])
                nc.sync.dma_start(out=be_s[p * 128:(p + 1) * 128, :], in_=beta[:])

                # q/k transposes -> kq pair layout + grp q/kT columns
                grp_n = pAc.tile([128, KT, 384], FPR, tag="grpn")
                kq_sb = pAc.tile([128, KT, 256], FPR, tag="kq")
                for jtb in range(KT // 4):
                    js = slice(jtb * 4, (jtb + 1) * 4)
                    psq = bigps.tile([128, 512], FPR, tag="big", name="psq")
                    psk = bigps.tile([128, 512], FPR, tag="big", name="psk")
                    for j4 in range(4):
                        jt = jtb * 4 + j4
                        nc.tensor.transpose(psq[:, j4 * 128:(j4 + 1) * 128],
                                            q_pair[:, jt * 128:(jt + 1) * 128],
                                            ident_r[:])
                        nc.tensor.transpose(psk[:, j4 * 128:(j4 + 1) * 128],
                                            k_pair[:, jt * 128:(jt + 1) * 128],
                                            ident_r[:])
                    psq4 = psq[:].rearrange("p (a b) -> p a b", b=128)
                    psk4 = psk[:].rearrange("p (a b) -> p a b", b=128)
                    nc.scalar.copy(out=kq_sb[:, js, 0:128], in_=psk4[:])
                    nc.scalar.copy(out=kq_sb[:, js, 128:256], in_=psq4[:])
                    for i, ko in ((0, _K0), (1, _K1)):
                        cs = slice(i * 64, (i + 1) * 64)
                        nc.scalar.copy(
                            out=grp_n[:, js, i * 128 + 64:i * 128 + 128],
                            in_=psq4[:, :, cs])
                        nc.scalar.copy(out=grp_n[:, js, ko:ko + 64],
                                       in_=psk4[:, :, cs])
                nc.sync.dma_start(out=kq_s[p], in_=kq_sb[:])
                # store q/kT columns (W columns are written by pass A2)
                nc.sync.dma_start(out=grp_s[p][:, :, 64:128], in_=grp_n[:, :, 64:128])
                nc.sync.dma_start(out=grp_s[p][:, :, 192:384], in_=grp_n[:, :, 192:384])

        # ============ pass A2: chunk-local solve (high parallelism) =========
        with tc.tile_pool(name="p2a", bufs=4) as p2a, \
             tc.tile_pool(name="pA2_ga", bufs=2, space="PSUM") as gaps2, \
             tc.tile_pool(name="pA2_lift", bufs=4, space="PSUM") as liftps, \
             tc.tile_pool(name="pA2_wp", bufs=2, space="PSUM") as wpps:
            for p in range(NPAIR):
                kq_l = p2a.tile([128, KT, 256], FPR, tag="kql")
                kn_l = p2a.tile([128, D], FPR, tag="knl")
                beta = p2a.tile([128, 1], FP, tag="betal")
                nc.sync.dma_start(out=kq_l[:], in_=kq_s[p])
                nc.sync.dma_start(out=kn_l[:], in_=kn_s[p * 128:(p + 1) * 128, :])
                nc.sync.dma_start(out=beta[:], in_=be_s[p * 128:(p + 1) * 128, :])

                gat = gaps2.tile([128, 256], FP, tag="ga", name="gat")
                for jt in range(KT):
                    nc.tensor.matmul(gat[:], kq_l[:, jt, 0:128], kq_l[:, jt, :],
                                     start=(jt == 0), stop=(jt == KT - 1))
                for i in range(2):
                    n = 2 * p + i
                    cs = slice(i * 64, (i + 1) * 64)
                    at_sb = p2a.tile([64, 64], FPR, tag=f"atsb{i}", name=f"atsb{i}")
                    nc.vector.tensor_tensor(out=at_sb[:],
                                            in0=gat[cs, 128 + i * 64:128 + i * 64 + 64],
                                            in1=mask_ui[:], op=AL.mult)
                    nc.sync.dma_start(out=at_s[n], in_=at_sb[:])

                for i in range(2):
                    n = 2 * p + i
                    cs = slice(i * 64, (i + 1) * 64)
                    kb_n = p2a.tile([64, D], FPR, tag=f"kb{i}", name=f"kb{i}")
                    nc.vector.tensor_scalar(out=kb_n[:], in0=kn_l[cs, :],
                                            scalar1=beta[cs, :], scalar2=None,
                                            op0=AL.mult)

                    _qstate = {"k": 4, "t": None}

                    def quarter(_qs=_qstate):
                        if _qs["k"] == 4:
                            _qs["t"] = liftps.tile([64, 256], FP, tag="lift",
                                                   name="lift_ps")
                            _qs["k"] = 0
                        q = _qs["t"][:, _qs["k"] * 64:(_qs["k"] + 1) * 64]
                        _qs["k"] += 1
                        return q

                    # L = -beta_col * G * strict_lower ; R = L^T
                    l_sb = p2a.tile([64, 64], FPR, tag=f"lsb{i}", name=f"lsb{i}")
                    nc.vector.scalar_tensor_tensor(out=l_sb[:],
                                                   in0=gat[cs, i * 64:(i + 1) * 64],
                                                   scalar=beta[cs, :],
                                                   in1=mask_l[cs, :],
                                                   op0=AL.mult, op1=AL.mult)
                    r_ps = quarter().bitcast(FPR)
                    nc.tensor.transpose(r_ps, l_sb[:], ident_r[0:64, 0:64])
                    r_sb = p2a.tile([64, 64], FPR, tag=f"rsb{i}", name=f"rsb{i}")
                    nc.scalar.copy(out=r_sb[:], in_=r_ps)

                    # binary lifting: T^T = prod_j (I + R^{2^j})
                    y_sb = p2a.tile([64, 64], FPR, tag=f"y0_{i}", name=f"y0_{i}")
                    nc.vector.tensor_tensor(out=y_sb[:], in0=r_ps,
                                            in1=ident_r[0:64, 0:64], op=AL.add)
                    p_sb, q_sb = l_sb, r_sb
                    for j in range(1, 6):
                        pp = quarter()
                        nc.tensor.matmul(pp, q_sb[:], p_sb[:], start=True, stop=True)
                        p_new = p2a.tile([64, 64], FPR, tag=f"p{j}_{i}",
                                         name=f"p{j}_{i}")
                        nc.scalar.copy(out=p_new[:], in_=pp)
                        if j < 5:
                            qp = quarter()
                            nc.tensor.matmul(qp, p_sb[:], q_sb[:], start=True,
                                             stop=True)
                            q_new = p2a.tile([64, 64], FPR, tag=f"q{j}_{i}",
                                             name=f"q{j}_{i}")
                            nc.scalar.copy(out=q_new[:], in_=qp)
                        else:
                            q_new = q_sb
                        yp = quarter()
                        nc.tensor.matmul(yp, p_new[:], y_sb[:], start=True, stop=True)
                        y_new = p2a.tile([64, 64], FPR, tag=f"y{j}_{i}",
                                         name=f"y{j}_{i}")
                        nc.vector.tensor_tensor(out=y_new[:], in0=yp, in1=y_sb[:],
                                                op=AL.add)
                        p_sb, q_sb, y_sb = p_new, q_new, y_new
                    tt_sb = y_sb  # T^T
                    nc.sync.dma_start(out=tt_s[n], in_=tt_sb[:])

                    # W^T blocks -> grp_s W columns
                    wps = wpps.tile([128, 512], FP, tag="wp", name="wps")
                    for jt in range(KT):
                        nc.tensor.matmul(wps[:, jt * 64:(jt + 1) * 64],
                                         kb_n[:, jt * 128:(jt + 1) * 128], tt_sb[:],
                                         start=True, stop=True)
                    wsb = p2a.tile([128, KT, 64], FPR, tag=f"wsb{i}", name=f"wsb{i}")
                    nc.vector.tensor_copy(
                        out=wsb[:],
                        in_=wps[:].rearrange("p (a b) -> p a b", b=64))
                    nc.sync.dma_start(out=grp_s[p][:, :, i * 128:i * 128 + 64],
                                      in_=wsb[:])

        # ================= pass B: V projection + U ========================
        if phases == 'a':
            continue
        with tc.tile_pool(name="wB", bufs=1) as wB, \
             tc.tile_pool(name="wnatB", bufs=2) as wnatB, \
             tc.tile_pool(name="pB", bufs=2) as pB, \
             tc.tile_pool(name="pBc", bufs=2) as pBc, \
             tc.tile_pool(name="pB_big", bufs=4, space="PSUM") as bigpsB, \
             tc.tile_pool(name="pB_u", bufs=2, space="PSUM") as upsB:
            wvT = [wB.tile([128, D], FPR, tag=f"wT_wv_{jt}", name=f"wT_wv_{jt}")
                   for jt in range(KT)]
            _transpose_weight(nc, wvT, w_d["wv"], wnatB, bigpsB, ident, "big")
            bias_v = _replicate_bias(nc, wB, b_d["wv"], "bias_v")

            for p in range(NPAIR):
                x_sb = pB.tile([128, D], FP, tag="x")
                nc.sync.dma_start(out=x_sb[:], in_=x_cur[p * 128:(p + 1) * 128, :])
                xt_sb = pB.tile([128, KT, 128], FPR, tag="xt")
                for jtb in range(KT // 4):
                    ps = bigpsB.tile([128, 512], FP, tag="big", name="xt_ps")
                    for j4 in range(4):
                        jt = jtb * 4 + j4
                        nc.tensor.transpose(ps[:, j4 * 128:(j4 + 1) * 128],
                                            x_sb[:, jt * 128:(jt + 1) * 128],
                                            ident[:])
                    nc.scalar.copy(
                        out=xt_sb[:, jtb * 4:(jtb + 1) * 4, :],
                        in_=ps[:].rearrange("p (a b) -> p a b", b=128))

                v_pair = pBc.tile([128, D], FPR, tag="vpair")
                for h in range(NH):
                    hs = slice(h * 512, (h + 1) * 512)
                    vp = bigpsB.tile([128, 512], FP, tag="big", name="vp")
                    for jt in range(KT):
                        nc.tensor.matmul(vp[:], xt_sb[:, jt, :], wvT[jt][:, hs],
                                         start=(jt == 0), stop=(jt == KT - 1))
                    nc.vector.tensor_tensor(out=v_pair[:, hs], in0=vp[:],
                                            in1=bias_v[:, hs], op=AL.add)

                beta = pBc.tile([128, 1], FP, tag="beta")
                nc.sync.dma_start(out=beta[:], in_=be_s[p * 128:(p + 1) * 128, :])

                for i in range(2):
                    n = 2 * p + i
                    cs = slice(i * 64, (i + 1) * 64)
                    vb_n = pBc.tile([64, D], FPR, tag=f"vb{i}", name=f"vb{i}")
                    nc.vector.tensor_scalar(out=vb_n[:], in0=v_pair[cs, :],
                                            scalar1=beta[cs, :], scalar2=None,
                                            op0=AL.mult)
                    tt_sb = pBc.tile([64, 64], FPR, tag=f"tt{i}", name=f"tt{i}")
                    nc.sync.dma_start(out=tt_sb[:], in_=tt_s[n])
                    u_ps = upsB.tile([64, D], FP, tag="u", name="u_ps")
                    for h in range(NH):
                        hs = slice(h * 512, (h + 1) * 512)
                        nc.tensor.matmul(u_ps[:, hs], tt_sb[:], vb_n[:, hs],
                                         start=True, stop=True)
                    u_sb = pBc.tile([64, D], FPR, tag=f"usb{i}", name=f"usb{i}")
                    nc.scalar.copy(out=u_sb[:], in_=u_ps[:])
                    nc.sync.dma_start(out=u_s[n], in_=u_sb[:])

        # ================= phase 2: the scan ================================
        if phases == 'ab':
            continue
        with tc.tile_pool(name="w2", bufs=1) as w2, \
             tc.tile_pool(name="wnat2", bufs=2) as wnat2, \
             tc.tile_pool(name="p2", bufs=2) as p2, \
             tc.tile_pool(name="p2s", bufs=1) as p2s, \
             tc.tile_pool(name="p2w", bufs=2) as p2w, \
             tc.tile_pool(name="p2_wqs", bufs=1, space="PSUM") as wqsp, \
             tc.tile_pool(name="p2_big", bufs=2, space="PSUM") as bps2, \
             tc.tile_pool(name="p2_sm", bufs=2, space="PSUM") as sps2:
            woT = [w2.tile([128, D], FPR, tag=f"wT_wo_{jt}", name=f"wT_wo_{jt}")
                   for jt in range(KT)]
            _transpose_weight(nc, woT, w_d["wo"], wnat2, bps2, ident, "big1024")
            bias_o = _replicate_bias(nc, w2, b_d["wo"], "bias_o")

            S_sb = p2s.tile([128, KT, D], FPR)
            zz = p2s.tile([128, D], FP)
            nc.vector.memset(zz[:], 0.0)
            for it in range(KT):
                nc.vector.tensor_copy(out=S_sb[:, it, :], in_=zz[:])

            for g in range(NPAIR):
                n0, n1 = 2 * g, 2 * g + 1
                grp_l = p2.tile([128, KT, 384], FPR, tag="grpl")
                u0_l = [p2.tile([64, D], FPR, tag=f"u0l{i}", name=f"u0l{i}")
                        for i in range(2)]
                k_pair = p2.tile([128, D], FPR, tag="kpair")
                nc.sync.dma_start(out=grp_l[:], in_=grp_s[g])
                for i, n in enumerate((n0, n1)):
                    nc.sync.dma_start(out=u0_l[i][:], in_=u_s[n])
                nc.sync.dma_start(out=k_pair[:], in_=kn_s[g * 128:(g + 1) * 128, :])

                ucat = p2w.tile([128, D], FPR, tag="ucat")
                u1n = p2w.tile([64, D], FPR, tag="u1n")
                ot_pair = p2w.tile([128, KT, 128], FPR, tag="otp")
                o_sb = [p2w.tile([64, D], FPR, tag=f"o{i}", name=f"o{i}")
                        for i in range(2)]

                at_sb = [p2.tile([64, 64], FPR, tag=f"at{i}", name=f"at{i}")
                         for i in range(2)]
                for i, n in enumerate((n0, n1)):
                    nc.sync.dma_start(out=at_sb[i][:], in_=at_s[n])
                # single product family: lhsT=kT0, rhs=[w1|q1|kT1] -> [cw|cq|.]
                bp0 = sps2.tile([64, 256], FP, tag="sm2", name="bp0")
                for jt in range(KT):
                    nc.tensor.matmul(bp0[:], grp_l[:, jt, _K0:_K0 + 64],
                                     grp_l[:, jt, _W1:_W1 + 256],
                                     start=(jt == 0), stop=(jt == KT - 1))
                cw_sb = p2w.tile([64, 64], FPR, tag="cw")
                cq_sb = p2w.tile([64, 64], FPR, tag="cq")
                nc.scalar.copy(out=cw_sb[:], in_=bp0[:, 0:64])
                nc.scalar.copy(out=cq_sb[:], in_=bp0[:, 64:128])

                for i in range(2):
                    wqs = wqsp.tile([128, D], FP, tag="wqs", name="wqs")
                    co = slice(i * 128, (i + 1) * 128)
                    for h in range(NH):
                        hs = slice(h * 512, (h + 1) * 512)
                        for it in range(KT):
                            nc.tensor.matmul(wqs[:, hs], grp_l[:, it, co],
                                             S_sb[:, it, hs], start=(it == 0),
                                             stop=(it == KT - 1 and i == 0))
                        if i == 1:
                            nc.tensor.matmul(wqs[0:64, hs], cw_sb[:], ucat[0:64, hs],
                                             start=False, stop=True)
                    udst = ucat[0:64, :] if i == 0 else u1n[:]
                    nc.vector.tensor_tensor(out=udst, in0=u0_l[i][:],
                                            in1=wqs[0:64, :], op=AL.subtract)
                    usrc = ucat[0:64, :] if i == 0 else u1n[:]
                    au = bps2.tile([64, D], FP, tag="big1024", name="au")
                    for h in range(NH):
                        hs = slice(h * 512, (h + 1) * 512)
                        nc.tensor.matmul(au[:, hs], at_sb[i][:], usrc[:, hs],
                                         start=True, stop=(i == 0))
                        if i == 1:
                            nc.tensor.matmul(au[:, hs], cq_sb[:], ucat[0:64, hs],
                                             start=False, stop=True)
                    nc.scalar.copy(out=o_sb[i][:], in_=wqs[64:128, :])
                    nc.vector.tensor_tensor(out=o_sb[i][:], in0=o_sb[i][:],
                                            in1=au[:], op=AL.add)
                    if i == 1:
                        nc.scalar.copy(out=ucat[64:128, :], in_=u1n[:])

                # group S update: S += kcat^T @ ucat (2 k-tiles via ACT+Pool)
                for it in range(KT):
                    sd = bps2.tile([128, D], FP, tag="big1024", name="sd")
                    for h in range(NH):
                        hs = slice(h * 512, (h + 1) * 512)
                        nc.tensor.matmul(sd[:, hs], k_pair[:, it * 128:(it + 1) * 128],
                                         ucat[:, hs], start=True, stop=True)
                    if it < 6:
                        nc.vector.tensor_tensor(out=S_sb[:, it, :], in0=S_sb[:, it, :],
                                                in1=sd[:], op=AL.add)
                    else:
                        sdc = p2w.tile([128, D], FP, tag="sdc", name="sdc", bufs=1)
                        nc.scalar.copy(out=sdc[:], in_=sd[:])
                        nc.gpsimd.tensor_tensor(out=S_sb[:, it, :], in0=S_sb[:, it, :],
                                                in1=sdc[:], op=AL.add)

                # transpose o chunks into ot_pair[:, jt, i*64:(i+1)*64]
                otp = bps2.tile([128, D], FPR, tag="big1024", name="otp")
                for i in range(2):
                    for jt in range(KT):
                        nc.tensor.transpose(otp[:, i * 512 + jt * 64:
                                                i * 512 + (jt + 1) * 64],
                                            o_sb[i][:, jt * 128:(jt + 1) * 128],
                                            ident_r[0:64, 0:64])
                nc.scalar.copy(
                    out=ot_pair[:].rearrange("p a (i b) -> p i a b", i=2),
                    in_=otp[:].rearrange("p (i a b) -> p i a b", i=2, b=64))

                # fused output projection
                op_ps = bps2.tile([128, D], FP, tag="big1024", name="op_ps")
                for h in range(NH):
                    hs = slice(h * 512, (h + 1) * 512)
                    for jt in range(KT):
                        nc.tensor.matmul(op_ps[:, hs], ot_pair[:, jt, :],
                                         woT[jt][:, hs],
                                         start=(jt == 0), stop=(jt == KT - 1))
                fo = p2w.tile([128, D], FP, tag="fo")
                nc.vector.tensor_tensor(out=fo[:], in0=op_ps[:], in1=bias_o[:],
                                        op=AL.add)
                nc.sync.dma_start(out=out_d[g * 128:(g + 1) * 128, :], in_=fo[:])

    nc.compile()
    return nc


def _get_nc(reps=1, phases='full'):
    key = f"nc{reps}_{phases}"
    if key not in _compiled:
        _compiled[key] = _build(reps, phases)
    return _compiled[key]


def _make_in_maps(inputs):
    X = np.ascontiguousarray(np.asarray(inputs["X"], dtype=np.float32))
    common = {
        "wq": np.ascontiguousarray(np.asarray(inputs["Wq_w"], np.float32)),
        "wk": np.ascontiguousarray(np.asarray(inputs["Wk_w"], np.float32)),
        "wv": np.ascontiguousarray(np.asarray(inputs["Wv_w"], np.float32)),
        "wo": np.ascontiguousarray(np.asarray(inputs["Wo_w"], np.float32)),
        "bq": np.ascontiguousarray(np.asarray(inputs["Wq_b"], np.float32).reshape(1, D)),
        "bk": np.ascontiguousarray(np.asarray(inputs["Wk_b"], np.float32).reshape(1, D)),
        "bv": np.ascontiguousarray(np.asarray(inputs["Wv_b"], np.float32).reshape(1, D)),
        "bo": np.ascontiguousarray(np.asarray(inputs["Wo_b"], np.float32).reshape(1, D)),
    }
    return [dict(common, x=np.ascontiguousarray(X[b])) for b in range(B)]


def kernel(X, chunk, Wq_w, Wq_b, Wk_w, Wk_b, Wv_w, Wv_b, Wo_w, Wo_b):
    nc = _get_nc()
    in_maps = _make_in_maps(dict(X=X, Wq_w=Wq_w, Wq_b=Wq_b, Wk_w=Wk_w, Wk_b=Wk_b,
                                 Wv_w=Wv_w, Wv_b=Wv_b, Wo_w=Wo_w, Wo_b=Wo_b))
    res = run_bass_kernel_spmd(nc, in_maps, list(range(B)))
    return np.stack([res.results[b]["out"] for b in range(B)], axis=0)

